# revision 1
# baseline (speedup 1.0000x reference)
"""GENConv-style message passing + MLP head on 8 trn2 NeuronCores.

Math restructuring (vs the reference):
  msg = relu(z) + eps, z = src_feat[src] + edge_attr @ w_edge.T
  softmax over each node's <=32 valid edges, out = sum(msg*alpha) + dst_feat.
  Because relu(z) >= 0 and |z| <~ 10, exp never overflows fp32, so the
  gather-max cancels analytically:
     S_n = sum_valid exp(relu(z)),  R_n = sum_valid relu(z)*exp(relu(z))
     out_n = R_n/S_n + eps + dst_feat_n
  with exp(relu(z)) = max(exp(z), 1) and relu(z)*exp(relu(z)) = relu(z*exp(z)).
  The "+eps" and "dst_feat @ w1.T" terms are per-channel-constant /  linear, so
  eps cancels through train-mode BatchNorm and dst is folded into the h-matmul
  with pre-multiplied weights w_d1 = w1 @ w_dst.

The per-edge source-feature gather commutes with the projection:
src_feat[src] = x[src] @ w_src.T, so the host stages x[src] per edge (bf16,
static indices - pure input rearrangement) and the device computes the K=128
projection itself; no indirect DMA is needed.  Invalid (padded) edges get a
host-solved mask column v with w_src @ v = -60*ones: they contribute exactly
1 to S (subtracted via a per-node count) and 0 to R.  Destination nodes are
sharded across 8 cores; BatchNorm batch stats are AllReduced.
"""

import math

import numpy as np
import ml_dtypes

# Problem constants (hardcoded per spec nn_ExportableGENConv_5377299054769).
N, K, IN_C, OUT_C, EDGE_D = 50000, 32, 128, 64, 32
H2 = 2 * OUT_C
NCORES = 8
BN_EPS = np.float32(1e-5)
MSG_EPS = np.float32(1e-7)
NEG_BIG = -60.0

BF16 = ml_dtypes.bfloat16


class Cfg:
    def __init__(self, cores, n_pc):
        self.cores = cores
        self.n_pc = n_pc                      # real nodes per core
        self.sup = math.ceil(n_pc / 128)      # supertiles (128 nodes / 4096 edges)
        self.n_pad = self.sup * 128
        self.e_pad = self.n_pad * K
        self.n_total = cores * n_pc


CFG = Cfg(NCORES, N // NCORES)


# --------------------------------------------------------------------------
# device program
# --------------------------------------------------------------------------

def build_nc(cfg: Cfg, debug: bool = False):
    import concourse.bass as bass
    import concourse.bacc as bacc
    import concourse.mybir as mybir
    import concourse.tile as tile

    dt = mybir.dt
    f32, bf, i32 = dt.float32, dt.bfloat16, dt.int32
    AF = mybir.ActivationFunctionType
    OP = mybir.AluOpType

    sup, n_pad, n_pc = cfg.sup, cfg.n_pad, cfg.n_pc
    cores = cfg.cores
    grp = [list(range(cores))]

    nc = bacc.Bacc("TRN2", num_devices=cores)

    eaT = nc.dram_tensor("eaT", [sup, 128, 1024], bf, kind="ExternalInput")
    xgT = nc.dram_tensor("xgT", [sup, 128, 4096], bf, kind="ExternalInput")
    xTd = nc.dram_tensor("xT", [128, n_pad], bf, kind="ExternalInput")
    corrd = nc.dram_tensor("corr", [128, sup * 64], bf, kind="ExternalInput")
    wedged = nc.dram_tensor("wedge", [128, 256], bf, kind="ExternalInput")
    wsrc2d = nc.dram_tensor("wsrc2", [128, 256], bf, kind="ExternalInput")
    w1eod = nc.dram_tensor("w1eo", [128, 256], bf, kind="ExternalInput")
    wd1Td = nc.dram_tensor("wd1T", [128, 128], bf, kind="ExternalInput")
    w2Td = nc.dram_tensor("w2T", [128, 64], bf, kind="ExternalInput")
    gamd = nc.dram_tensor("gam", [128, 1], f32, kind="ExternalInput")
    betd = nc.dram_tensor("bet", [128, 1], f32, kind="ExternalInput")
    identfd = nc.dram_tensor("identf", [128, 128], f32, kind="ExternalInput")
    yout = nc.dram_tensor("yout", [128, sup * 64], f32, kind="ExternalOutput")
    if debug:
        dbg_S = nc.dram_tensor("dbg_S", [128, sup * 64], bf, kind="ExternalOutput")
        dbg_R = nc.dram_tensor("dbg_R", [128, sup * 64], bf, kind="ExternalOutput")
        dbg_h = nc.dram_tensor("dbg_h", [128, n_pad], bf, kind="ExternalOutput")

    n_chunks = math.ceil(n_pad / 512)

    with tile.TileContext(nc) as tc:
        with (
            tc.tile_pool(name="dram", bufs=1, space="DRAM") as dpool,
            tc.tile_pool(name="const", bufs=1) as cpool,
            tc.tile_pool(name="work", bufs=2) as wpool,
        ):
            # ---- persistent SBUF state ----
            xT = cpool.tile([128, n_pad], bf)
            corr_sb = cpool.tile([128, sup * 64], bf)
            we2 = cpool.tile([128, 256], bf)
            wsrc2 = cpool.tile([128, 256], bf)
            w1eo = cpool.tile([128, 256], bf)
            wd1T = cpool.tile([128, 128], bf)
            w2T = cpool.tile([128, 64], bf)
            gam = cpool.tile([128, 1], f32)
            bet = cpool.tile([128, 1], f32)
            identf = cpool.tile([128, 128], f32)
            S_all = cpool.tile([128, sup * 64], bf)
            R_all = cpool.tile([128, sup * 64], bf)
            recip = cpool.tile([128, sup * 64], bf)
            h_sb = cpool.tile([128, n_pad], bf)
            y_sb2 = cpool.tile([128, sup * 64], f32)
            hsum = cpool.tile([128, n_chunks], f32)
            sqsum = cpool.tile([128, n_chunks], f32)
            bn_sb = cpool.tile([128, 2], f32)
            bn2_sb = cpool.tile([128, 2], f32)
            stat = cpool.tile([128, 8], f32)  # mean|msq|var|rvar|rstd|scale|shift|tmp

            for dst_t, src_t in (
                (xT, xTd), (corr_sb, corrd), (we2, wedged), (wsrc2, wsrc2d),
                (w1eo, w1eod), (wd1T, wd1Td), (w2T, w2Td),
                (gam, gamd), (bet, betd), (identf, identfd),
            ):
                nc.sync.dma_start(out=dst_t[:], in_=src_t[:])

            bn_in = dpool.tile([128, 2], f32)
            bn_out = dpool.tile([128, 2], f32)

            # ---- edge phase ----
            with tc.tile_pool(name="z", bufs=2, space="PSUM") as zpool:
                for s in range(sup):
                    ea_sb = wpool.tile([128, 1024], bf, tag="ea", bufs=3)
                    nc.sync.dma_start(out=ea_sb[:], in_=eaT[s])
                    xg_sb = wpool.tile([128, 4096], bf, tag="xg", bufs=3)
                    nc.sync.dma_start(out=xg_sb[:], in_=xgT[s])
                    z = zpool.tile([128, 2048], f32, tag="z")
                    # src projection, stack A then stack B (zero-padded lhsT)
                    for b in range(4):
                        nc.tensor.matmul(
                            out=z[:, 512 * b:512 * b + 512],
                            lhsT=wsrc2[:, 0:128],
                            rhs=xg_sb[:, 1024 * b:1024 * b + 512],
                            start=True, stop=False, skip_group_check=True,
                        )
                    for b in range(4):
                        nc.tensor.matmul(
                            out=z[:, 512 * b:512 * b + 512],
                            lhsT=wsrc2[:, 128:256],
                            rhs=xg_sb[:, 1024 * b + 512:1024 * b + 1024],
                            start=False, stop=False, skip_group_check=True,
                        )
                    # edge-attr projection (block-diagonal, both stacks at once)
                    for b in range(4):
                        nc.tensor.matmul(
                            out=z[:, 512 * b:512 * b + 512],
                            lhsT=we2[:, 128 * (b // 2):128 * (b // 2) + 128],
                            rhs=ea_sb[:, 512 * (b % 2):512 * (b % 2) + 512],
                            start=False, stop=True, skip_group_check=True,
                        )
                    w0 = wpool.tile([128, 2048], bf, tag="w0", bufs=2)
                    nc.scalar.activation(out=w0[:], in_=z[:], func=AF.Exp)
                    zc = wpool.tile([128, 2048], bf, tag="zc", bufs=2)
                    nc.scalar.activation(out=zc[:], in_=z[:], func=AF.Copy)
                    tt = wpool.tile([128, 2048], bf, tag="tt", bufs=2)
                    nc.vector.tensor_tensor(out=tt[:], in0=zc[:], in1=w0[:], op=OP.mult)
                    nc.vector.tensor_scalar_max(out=w0[:], in0=w0[:], scalar1=1.0)
                    nc.vector.tensor_scalar_max(out=tt[:], in0=tt[:], scalar1=0.0)
                    for buf, dstall in ((w0, S_all), (tt, R_all)):
                        w = 16
                        while w >= 1:
                            vin = buf[:, :64 * 2 * w].rearrange(
                                "p (n k) -> p n k", k=2 * w)
                            if w > 1:
                                vout = buf[:, :64 * w].rearrange(
                                    "p (n k) -> p n k", k=w)
                            else:
                                vout = dstall[:, 64 * s:64 * s + 64].rearrange(
                                    "p (n k) -> p n k", k=1)
                            nc.vector.tensor_tensor(
                                out=vout,
                                in0=vin[:, :, 0:w], in1=vin[:, :, w:2 * w],
                                op=OP.add,
                            )
                            w //= 2

            # ---- finalize out = R/S (stacked layout) ----
            nc.vector.tensor_tensor(out=S_all[:], in0=S_all[:], in1=corr_sb[:],
                                    op=OP.subtract)
            with nc.allow_low_precision(reason="bf16 softmax denom; 2e-2 gate"):
                nc.vector.reciprocal(out=recip[:], in_=S_all[:])
            nc.vector.tensor_tensor(out=R_all[:], in0=R_all[:], in1=recip[:],
                                    op=OP.mult)

            # ---- MLP head ----
            with (
                tc.tile_pool(name="hp", bufs=2, space="PSUM") as hpool,
                tc.tile_pool(name="yp", bufs=2, space="PSUM") as ypool,
                tc.tile_pool(name="ytp", bufs=2, space="PSUM") as ytpool,
            ):
                for cc in range(n_chunks):
                    c0 = 512 * cc
                    cw = min(512, n_pad - c0)
                    h_ps = hpool.tile([128, 512], f32, tag="hp")
                    qs = [q for q in range(cw // 64) if (8 * cc + q) % 2 == 0] + \
                         [q for q in range(cw // 64) if (8 * cc + q) % 2 == 1]
                    for j, q in enumerate(qs):
                        g = 8 * cc + q
                        s_, a_ = g // 2, g % 2
                        nc.tensor.matmul(
                            out=h_ps[:, 64 * q:64 * q + 64],
                            lhsT=w1eo[:, 128 * a_:128 * a_ + 128],
                            rhs=R_all[:, 64 * s_:64 * s_ + 64],
                            start=(j == 0), stop=False,
                            skip_group_check=True,
                        )
                    nc.tensor.matmul(
                        out=h_ps[:, :cw], lhsT=wd1T[:], rhs=xT[:, c0:c0 + cw],
                        start=False, stop=True, skip_group_check=True,
                    )
                    # copy h -> SBUF while accumulating batch stats.  pad
                    # nodes have h == 0 exactly (out=0, x=0) so summing all
                    # columns still yields the real-node sums.
                    nc.scalar.activation(
                        out=h_sb[:, c0:c0 + cw], in_=h_ps[:, :cw],
                        func=AF.Copy, accum_out=hsum[:, cc:cc + 1])
                    sq = wpool.tile([128, 512], bf, tag="sq", bufs=2)
                    nc.scalar.activation(
                        out=sq[:, :cw], in_=h_ps[:, :cw],
                        func=AF.Square, accum_out=sqsum[:, cc:cc + 1])

                nc.vector.tensor_reduce(out=bn_sb[:, 0:1], in_=hsum[:],
                                        axis=mybir.AxisListType.X, op=OP.add)
                nc.vector.tensor_reduce(out=bn_sb[:, 1:2], in_=sqsum[:],
                                        axis=mybir.AxisListType.X, op=OP.add)
                nc.sync.dma_start(out=bn_in[:], in_=bn_sb[:])
                nc.gpsimd.collective_compute(
                    "AllReduce", OP.add, replica_groups=grp,
                    ins=[bn_in[:].opt()], outs=[bn_out[:].opt()],
                )
                nc.sync.dma_start(out=bn2_sb[:], in_=bn_out[:])

                inv_n = 1.0 / float(cfg.n_total)
                mean, msq, var, rvar, rstd, scale, shift, tmp = (
                    stat[:, i:i + 1] for i in range(8))
                nc.vector.tensor_scalar_mul(out=mean, in0=bn2_sb[:, 0:1], scalar1=inv_n)
                nc.vector.tensor_scalar_mul(out=msq, in0=bn2_sb[:, 1:2], scalar1=inv_n)
                nc.vector.tensor_tensor(out=tmp, in0=mean, in1=mean, op=OP.mult)
                nc.vector.tensor_tensor(out=var, in0=msq, in1=tmp, op=OP.subtract)
                nc.vector.tensor_scalar_add(out=var, in0=var, scalar1=float(BN_EPS))
                nc.vector.reciprocal(out=rvar, in_=var)
                nc.scalar.activation(out=rstd, in_=rvar, func=AF.Sqrt)
                nc.vector.tensor_tensor(out=scale, in0=gam[:], in1=rstd, op=OP.mult)
                nc.vector.tensor_tensor(out=tmp, in0=mean, in1=scale, op=OP.mult)
                nc.vector.tensor_tensor(out=shift, in0=bet[:], in1=tmp, op=OP.subtract)

                nc.vector.tensor_scalar(out=h_sb[:], in0=h_sb[:],
                                        scalar1=scale, scalar2=shift,
                                        op0=OP.mult, op1=OP.add)
                nc.vector.tensor_scalar_max(out=h_sb[:], in0=h_sb[:], scalar1=0.0)

                for cc in range(n_chunks):
                    c0 = 512 * cc
                    cw = min(512, n_pad - c0)
                    y_ps = ypool.tile([64, 512], f32, tag="yp")
                    nc.tensor.matmul(out=y_ps[:, :cw], lhsT=w2T[:],
                                     rhs=h_sb[:, c0:c0 + cw],
                                     start=True, stop=True)
                    y_sc = wpool.tile([64, 512], f32, tag="ysc", bufs=2)
                    nc.scalar.activation(out=y_sc[:, :cw], in_=y_ps[:, :cw],
                                         func=AF.Copy)
                    for q in range(cw // 128):
                        yt_ps = ytpool.tile([128, 64], f32, tag="ytp")
                        nc.tensor.matmul(
                            out=yt_ps[:], lhsT=y_sc[:, 128 * q:128 * q + 128],
                            rhs=identf[0:64, 0:64], is_transpose=True,
                            start=True, stop=True, skip_group_check=True,
                        )
                        g = 4 * cc + q
                        nc.vector.tensor_copy(out=y_sb2[:, 64 * g:64 * g + 64],
                                              in_=yt_ps[:])
                nc.sync.dma_start(out=yout[:], in_=y_sb2[:])
                if debug:
                    nc.sync.dma_start(out=dbg_S[:], in_=S_all[:])
                    nc.sync.dma_start(out=dbg_R[:], in_=R_all[:])
                    nc.sync.dma_start(out=dbg_h[:], in_=h_sb[:])

    nc.finalize()
    return nc


# --------------------------------------------------------------------------
# host side
# --------------------------------------------------------------------------

def _perm(cfg: Cfg):
    """device column order -> local node id.
    col c = 64*(2s+a)+i  ->  node 128s + 32*(i//16) + 16a + (i%16)"""
    c = np.arange(cfg.n_pad)
    g, i = c // 64, c % 64
    s, a = g // 2, g % 2
    return 128 * s + 32 * (i // 16) + 16 * a + (i % 16)


def host_inputs(cfg: Cfg, x, edge_attr, w_src, w_dst, w_edge, w1, gamma, beta,
                w2, src, valid):
    """Build per-core in_maps. src: [E] int64 global src node per edge,
    valid: [Ntot, K] bool."""
    n_pc, sup, n_pad, e_pad = cfg.n_pc, cfg.sup, cfg.n_pad, cfg.e_pad
    e_pc = n_pc * K

    perm = _perm(cfg)                       # device col -> local node

    # device edge order: eh = 4096s + 1024b + 512k + jj
    #   node-in-half q = 16b + jj//32, half k, edge-in-node kk = jj%32
    eh = np.arange(e_pad)
    s_e = eh // 4096
    r4 = eh % 4096
    b_e = r4 // 1024
    k_e = (r4 % 1024) // 512
    jj = r4 % 512
    q_e = 16 * b_e + jj // 32
    ch = 64 * (2 * s_e + k_e) + q_e         # device col of the edge's node
    pn = perm[ch]                           # local (padded) node id
    le = 32 * pn + jj % 32                  # local padded edge id

    # mask column: w_src @ v_mask = NEG_BIG * ones
    v_mask = np.linalg.lstsq(w_src.astype(np.float64),
                             np.full(OUT_C, NEG_BIG, np.float64), rcond=None)[0]
    v_mask = v_mask.astype(np.float32)

    # blockdiag proj weights: top = [[diag2(wT)],[0]], bot = [[0],[diag2(wT)]]
    wT = np.ascontiguousarray(w_edge.T).astype(np.float32)      # [32, 64]
    d2 = np.zeros((64, 128), np.float32)
    d2[0:32, 0:64] = wT
    d2[32:64, 64:128] = wT
    wedge = np.zeros((128, 256), np.float32)
    wedge[0:64, 0:128] = d2
    wedge[64:128, 128:256] = d2
    wedge = wedge.astype(BF16)

    wsT = np.ascontiguousarray(w_src.T).astype(np.float32)      # [128, 64]
    wsrc2 = np.zeros((128, 256), np.float32)
    wsrc2[:, 0:64] = wsT          # wsA: out partitions 0:64
    wsrc2[:, 128 + 64:256] = wsT  # wsB: out partitions 64:128
    wsrc2 = wsrc2.astype(BF16)
    w1T = np.ascontiguousarray(w1.T).astype(np.float32)         # [64, 128]
    w1eo = np.zeros((128, 256), np.float32)
    w1eo[0:64, 0:128] = w1T      # even halves: data on partitions 0:64
    w1eo[64:128, 128:256] = w1T  # odd halves
    w1eo = w1eo.astype(BF16)
    wd1T = np.ascontiguousarray((w1 @ w_dst).T).astype(BF16)
    w2T = np.ascontiguousarray(w2.T).astype(BF16)
    identf = np.eye(128, dtype=np.float32)
    gam = gamma.reshape(128, 1).astype(np.float32)
    bet = beta.reshape(128, 1).astype(np.float32)

    deg = valid.sum(axis=1)

    in_maps = []
    for c in range(cfg.cores):
        n0 = c * n_pc
        e0 = n0 * K
        eap = np.zeros((e_pad, EDGE_D), np.float32)
        eap[:e_pc] = edge_attr[e0:e0 + e_pc]
        ea_dev = eap[le]                                   # [e_pad, 32]
        # eaT_b[s, 64*(b//2) + 32k + d, 512*(b%2) + jj] = ea_dev[eh, d]
        ea5 = ea_dev.reshape(sup, 4, 2, 512, EDGE_D)       # s, b, k, jj, d
        eaT_b = np.zeros((sup, 128, 1024), np.float32)
        for b in range(4):
            blk = ea5[:, b].transpose(0, 1, 3, 2)          # s, k, d, jj
            eaT_b[:, 64 * (b // 2):64 * (b // 2) + 64,
                  512 * (b % 2):512 * (b % 2) + 512] = blk.reshape(sup, 64, 512)
        eaT_b = eaT_b.astype(BF16)

        vm = np.zeros(e_pad, bool)
        real = pn < n_pc
        vm[real] = valid[n0:n0 + n_pc].reshape(-1)[le[real]]
        xg_dev = np.empty((e_pad, IN_C), np.float32)
        xg_dev[:] = v_mask
        xg_dev[vm] = x[src[e0 + le[vm]]]
        xgT_b = np.ascontiguousarray(
            xg_dev.reshape(sup, 4096, IN_C).transpose(0, 2, 1)).astype(BF16)

        xpad = np.zeros((n_pad, IN_C), np.float32)
        xpad[:n_pc] = x[n0:n0 + n_pc]
        xTc = np.ascontiguousarray(xpad[perm].T).astype(BF16)   # [128, n_pad]

        cnt = np.full(n_pad, float(K - 1), np.float32)
        cnt[:n_pc] = (K - deg[n0:n0 + n_pc]).astype(np.float32)
        cntp = cnt[perm].reshape(sup, 2, 64)               # s, a, i
        corr = np.empty((128, sup * 64), np.float32)
        corr[:64] = cntp[:, 0, :].reshape(-1)
        corr[64:] = cntp[:, 1, :].reshape(-1)
        corr = corr.astype(BF16)

        in_maps.append({
            "eaT": eaT_b, "xgT": xgT_b, "xT": xTc, "corr": corr,
            "wedge": wedge, "wsrc2": wsrc2, "w1eo": w1eo, "wd1T": wd1T,
            "w2T": w2T, "gam": gam, "bet": bet, "identf": identf,
        })
    return in_maps


def assemble_output(cfg: Cfg, results):
    perm = _perm(cfg)
    outs = []
    for c in range(cfg.cores):
        y = np.asarray(results[c]["yout"], np.float32)       # [128, sup*64]
        y = y.reshape(128, cfg.sup, 64).transpose(1, 0, 2).reshape(cfg.n_pad, 64)
        yo = np.empty((cfg.n_pad, 64), np.float32)
        yo[perm] = y                                         # device row r holds node perm[r]
        outs.append(yo[:cfg.n_pc])
    return np.concatenate(outs, axis=0)


_CACHE = {}
TRACE = False        # set by test harness to capture a HW profile
DEBUG = False        # build with extra debug outputs
LAST_RESULT = None   # BassKernelResults of the last run (for exec_time_ns)


def kernel(x, edge_attr, w_src, w_dst, w_edge, w1, gamma, beta, w2, edge_index,
           nbr):
    from concourse import bass_utils

    x = np.asarray(x, np.float32)
    edge_attr = np.asarray(edge_attr, np.float32)
    w_src = np.asarray(w_src, np.float32)
    w_dst = np.asarray(w_dst, np.float32)
    w_edge = np.asarray(w_edge, np.float32)
    w1 = np.asarray(w1, np.float32)
    gamma = np.asarray(gamma, np.float32)
    beta = np.asarray(beta, np.float32)
    w2 = np.asarray(w2, np.float32)
    edge_index = np.asarray(edge_index)
    nbr = np.asarray(nbr)

    src = edge_index[0].astype(np.int64)
    valid = nbr >= 0
    # the kernel relies on the contiguous-edge-block structure of the graph
    E = N * K
    assert (edge_index[1] == np.repeat(np.arange(N, dtype=np.int64), K)).all()
    ar = np.arange(E, dtype=np.int64).reshape(N, K)
    assert ((nbr < 0) | (nbr == ar)).all()

    cfg = CFG
    in_maps = host_inputs(cfg, x, edge_attr, w_src, w_dst, w_edge, w1, gamma,
                          beta, w2, src, valid)
    if "nc" not in _CACHE:
        _CACHE["nc"] = build_nc(cfg, debug=DEBUG)
    res = bass_utils.run_bass_kernel_spmd(
        _CACHE["nc"], in_maps, core_ids=list(range(cfg.cores)), trace=TRACE)
    global LAST_RESULT
    LAST_RESULT = res
    return assemble_output(cfg, res.results)

